# revision 1
# baseline (speedup 1.0000x reference)
"""Trainium2 Bass kernel for nn_Net_6maxFull (batch of tiny LSTM chains).

Strategy (pure data parallel over 8 cores, batch on the free axis):
  - 30 LSTM cells scheduled into 12 "slots" of up to 3 independent cells.
  - Per slot one block-diagonal matmul computes all gates:
      lhsT [K, M] host-packed: K = [h-chain rows | x/state rows],
      M = gate rows grouped 32-aligned: i@0, f@32, o@64, g@96.
  - Biases folded into ScalarE activation bias APs.
  - sigmoid(i,f,o) in one ACT instr, tanh(g) in one, c2-tanh in one.
  - c2 = f*c + i*g via two DVE ops using 32-aligned partition groups.
  - h written straight into the next slot's matmul rhs tile; copies of h
    into head-concat tiles go over SBUF->SBUF DMA.
  - Heads (W1/W1o/W2/W3) as small matmuls at end of each batch tile.
"""
import sys
import numpy as np

sys.path.insert(0, "/opt/trn_rl_repo")

B = 131072
NCORE = 8
BC = B // NCORE
H = 10

# slot schedule: list of cells; cell = ("g", layer) or ("o", branch, step)
SLOTS = (
    [[("g", 0), ("o", 0, 0), ("o", 1, 0)],
     [("g", 1), ("o", 0, 1), ("o", 1, 1)],
     [("g", 2), ("o", 0, 2), ("o", 1, 2)],
     [("g", 3), ("o", 0, 3), ("o", 1, 3)],
     [("g", 4), ("o", 2, 0), ("o", 3, 0)],
     [("g", 5), ("o", 2, 1), ("o", 3, 1)],
     [("g", 6), ("o", 2, 2), ("o", 3, 2)],
     [("g", 7), ("o", 2, 3), ("o", 3, 3)],
     [("g", 8), ("o", 4, 0)],
     [("g", 9), ("o", 4, 1)],
     [("o", 4, 2)],
     [("o", 4, 3)]]
)
NSLOT = len(SLOTS)

# gate group partition offsets inside the gates psum/sbuf tiles
GI, GF, GO, GG = 0, 32, 64, 96


def _is_start(cell):
    return (cell[0] == "g" and cell[1] == 0) or (cell[0] == "o" and cell[2] == 0)


def _pred(cell):
    if cell[0] == "g":
        return ("g", cell[1] - 1)
    return ("o", cell[1], cell[2] - 1)


def _x_rows(cell):
    # row range of x^T feeding a chain-start cell
    if cell[0] == "g":
        return (0, 12)
    p = cell[1]
    s = 12 + 5 * p + 1
    return (s, s + 4)


class Plan:
    """Host-side packing plan: row layouts of st/ct blocks and lhsT maps."""

    def __init__(self):
        self.slot = []
        for t, cells in enumerate(SLOTS):
            info = {"cells": cells, "nc": len(cells)}
            info["hp"] = 0 if t == 0 else 10 * len(SLOTS[t - 1])
            # DMA block rows: x rows for start cells then h-state rows per cell
            rows = []  # list of (kind, cell) kind in {x, h}
            for c in cells:
                if _is_start(c):
                    rows.append(("x", c))
            for c in cells:
                rows.append(("h", c))
            info["strows"] = rows
            info["R"] = sum(4 if (k == "x" and c[0] == "o") else
                            12 if (k == "x") else 10 for k, c in rows)
            info["K"] = info["hp"] + info["R"]
            info["M"] = 128  # g-group padded to full 32 rows
            self.slot.append(info)


PLAN = Plan()


def pack_host(inp, np_dt):
    """Build all DRAM-side arrays (full batch; sharding happens later).

    Returns dict name -> np.ndarray. Batch-carrying arrays have shape
    [rows, B]; weights/bias arrays are replicated across cores.
    """
    f32 = np.float32
    out = {}
    Bt = inp["x"].shape[0]
    xT = np.ascontiguousarray(inp["x"].T.astype(np_dt))            # [37, B]
    genh = {i: np.ascontiguousarray(inp["gen_h"][i].T.astype(np_dt)) for i in range(10)}
    genc = {i: np.ascontiguousarray(inp["gen_c"][i].T.astype(np_dt)) for i in range(10)}
    opph = {(p, s): np.ascontiguousarray(inp["opp_h"][p][s].T.astype(np_dt))
            for p in range(5) for s in range(4)}
    oppc = {(p, s): np.ascontiguousarray(inp["opp_c"][p][s].T.astype(np_dt))
            for p in range(5) for s in range(4)}

    def cell_w(cell):
        # returns Wih [40, din], Whh [40, 10], bias [40]
        if cell[0] == "g":
            i = cell[1]
            if i == 0:
                return (inp["W_g0_ih"], inp["W_g0_hh"],
                        inp["b_g0_ih"] + inp["b_g0_hh"])
            return (inp["W_g_ih"][i - 1], inp["W_g_hh"][i - 1],
                    inp["b_g_ih"][i - 1] + inp["b_g_hh"][i - 1])
        p, s = cell[1], cell[2]
        if s == 0:
            return (inp["W_o0_ih"][p], inp["W_o0_hh"][p],
                    inp["b_o0_ih"][p] + inp["b_o0_hh"][p])
        return (inp["W_o_ih"][p][s - 1], inp["W_o_hh"][p][s - 1],
                inp["b_o_ih"][p][s - 1] + inp["b_o_hh"][p][s - 1])

    for t, info in enumerate(PLAN.slot):
        cells = info["cells"]
        # ---- st block [R, B] ----
        st = np.empty((info["R"], Bt), np_dt)
        row_of = {}
        r = 0
        for kind, c in info["strows"]:
            if kind == "x":
                a, b = _x_rows(c)
                st[r:r + (b - a)] = xT[a:b]
                row_of[("x", c)] = r
                r += b - a
            else:
                src = genh[c[1]] if c[0] == "g" else opph[(c[1], c[2])]
                st[r:r + 10] = src
                row_of[("h", c)] = r
                r += 10
        out[f"st{t}"] = st
        # ---- ct block [10*nc, B] ----
        ct = np.empty((10 * info["nc"], Bt), np_dt)
        for k, c in enumerate(cells):
            src = genc[c[1]] if c[0] == "g" else oppc[(c[1], c[2])]
            ct[10 * k:10 * k + 10] = src
        out[f"ct{t}"] = ct
        # ---- lhsT [K, M] and bias [128] ----
        lw = np.zeros((info["K"], info["M"]), f32)
        bias = np.zeros((128, 1), f32)
        prev_cells = SLOTS[t - 1] if t > 0 else []
        for k, c in enumerate(cells):
            Wih, Whh, bvec = cell_w(c)
            Wih = np.asarray(Wih, f32)
            Whh = np.asarray(Whh, f32)
            bvec = np.asarray(bvec, f32)
            # gate row slices in torch order i,f,g,o
            gslice = {"i": slice(0, 10), "f": slice(10, 20),
                      "g": slice(20, 30), "o": slice(30, 40)}
            goff = {"i": GI + 10 * k, "f": GF + 10 * k,
                    "o": GO + 10 * k, "g": GG + 10 * k}
            # tanh-everywhere: sigmoid(x) = (tanh(x/2)+1)/2, so pre-acts of
            # i,f,o are halved; chained h inputs carry h' = 2h, so those
            # columns get an extra 0.5.
            gsc = {"i": 0.5, "f": 0.5, "o": 0.5, "g": 1.0}
            if _is_start(c):
                r0 = info["hp"] + row_of[("x", c)]
                din = Wih.shape[1]
                for gn in "ifog":
                    lw[r0:r0 + din, goff[gn]:goff[gn] + 10] = gsc[gn] * Wih[gslice[gn]].T
            else:
                pos = prev_cells.index(_pred(c))
                r0 = 10 * pos
                for gn in "ifog":
                    lw[r0:r0 + 10, goff[gn]:goff[gn] + 10] = 0.5 * gsc[gn] * Wih[gslice[gn]].T
            # state rows
            r0 = info["hp"] + row_of[("h", c)]
            for gn in "ifog":
                lw[r0:r0 + 10, goff[gn]:goff[gn] + 10] = gsc[gn] * Whh[gslice[gn]].T
                bias[goff[gn]:goff[gn] + 10, 0] = gsc[gn] * bvec[gslice[gn]]
        out[f"lw{t}"] = lw.astype(np_dt)
        out[f"bias{t}"] = bias

    # ---- heads ----
    W1 = np.asarray(inp["W1"], f32)      # [50, 100]
    W1o = np.asarray(inp["W1o"], f32)    # [20, 40]
    W2 = np.asarray(inp["W2"], f32)      # [10, 70]
    W3 = np.asarray(inp["W3"], f32)      # [1, 10]
    out["whg"] = (0.5 * W1.T).copy().astype(np_dt)       # [100, 50]
    who = np.zeros((80, 40), f32)
    for s in range(4):
        blk = 0.5 * W1o[:, 10 * s:10 * s + 10].T         # [10, 20]
        who[20 * s:20 * s + 10, 0:20] = blk
        who[20 * s + 10:20 * s + 20, 20:40] = blk
    out["who01"] = who.astype(np_dt)
    out["who23"] = who.astype(np_dt)
    out["who4"] = (0.5 * W1o.T).copy().astype(np_dt)     # [40, 20]
    out["w2a"] = W2[:, 0:50].T.copy().astype(np_dt)      # [50, 10]
    w2o = (W2[:, 50:70] / 5.0).T                          # [20, 10]
    out["w2b"] = np.vstack([w2o, w2o]).astype(np_dt)     # [40, 10]
    out["w2c"] = w2o.copy().astype(np_dt)                # [20, 10]
    out["w3"] = W3.T.copy().astype(np_dt)                # [10, 1]
    for w_ in (10, 20, 30):
        ia = np.zeros((GF + w_, w_), f32)
        for r in range(w_):
            ia[r, r] = 0.5
            ia[GF + r, r] = 0.5
        out[f"iadd{w_}"] = ia.astype(np_dt)
    hb = np.zeros((128, 8), f32)
    hb[0:50, 0] = np.asarray(inp["b1"], f32)
    hb[0:40, 1] = np.tile(np.asarray(inp["b1o"], f32), 2)
    hb[0:20, 2] = np.asarray(inp["b1o"], f32)
    hb[0:10, 3] = np.asarray(inp["b2"], f32)
    hb[0:1, 4] = np.asarray(inp["b3"], f32)
    out["hbias"] = hb
    return out


def build_nc(Bc, FD, np_dt):
    """Build the SPMD Bass program for one core over Bc batch columns."""
    import concourse.bass as bass
    import concourse.tile as tile
    from concourse import bacc, mybir

    dt = {np.dtype(np.float32): mybir.dt.float32}.get(np.dtype(np_dt))
    if dt is None:
        import ml_dtypes
        assert np.dtype(np_dt) == np.dtype(ml_dtypes.bfloat16)
        dt = mybir.dt.bfloat16
    f32 = mybir.dt.float32
    AF = mybir.ActivationFunctionType

    PSUM_FD = min(1024, FD)
    N_MM = min(512, PSUM_FD)
    n_tiles = Bc // FD
    assert Bc % FD == 0 and FD % PSUM_FD == 0 and PSUM_FD % N_MM == 0

    nc = bacc.Bacc(None, target_bir_lowering=False, debug=False)
    P = PLAN.slot
    dr = {}
    for t in range(NSLOT):
        dr[f"st{t}"] = nc.declare_dram_parameter(f"st{t}", [P[t]["R"], Bc], dt, isOutput=False)
        dr[f"ct{t}"] = nc.declare_dram_parameter(f"ct{t}", [10 * P[t]["nc"], Bc], dt, isOutput=False)
        dr[f"lw{t}"] = nc.declare_dram_parameter(f"lw{t}", [P[t]["K"], P[t]["M"]], dt, isOutput=False)
        dr[f"bias{t}"] = nc.declare_dram_parameter(f"bias{t}", [128, 1], f32, isOutput=False)
    for name, shp in [("whg", [100, 50]), ("who01", [80, 40]), ("who23", [80, 40]),
                      ("who4", [40, 20]), ("w2a", [50, 10]), ("w2b", [40, 10]),
                      ("w2c", [20, 10]), ("w3", [10, 1]),
                      ("iadd10", [42, 10]), ("iadd20", [52, 20]), ("iadd30", [62, 30])]:
        dr[name] = nc.declare_dram_parameter(name, shp, dt, isOutput=False)
    dr["hbias"] = nc.declare_dram_parameter("hbias", [128, 8], f32, isOutput=False)
    out_d = nc.declare_dram_parameter("out", [1, Bc], f32, isOutput=True)

    from contextlib import ExitStack
    with tile.TileContext(nc) as tc:
        with ExitStack() as ctx:
            consts = ctx.enter_context(tc.tile_pool(name="consts", bufs=1))
            rhsp = ctx.enter_context(tc.tile_pool(name="rhs", bufs=7))
            sp = ctx.enter_context(tc.tile_pool(name="sig", bufs=4))
            zp = ctx.enter_context(tc.tile_pool(name="z", bufs=4))
            up = ctx.enter_context(tc.tile_pool(name="u", bufs=3))
            cp = ctx.enter_context(tc.tile_pool(name="c2", bufs=3))
            hp_ = ctx.enter_context(tc.tile_pool(name="hcat", bufs=2))
            fp = ctx.enter_context(tc.tile_pool(name="fh", bufs=1))
            outp = ctx.enter_context(tc.tile_pool(name="outp", bufs=2))
            pg = ctx.enter_context(tc.tile_pool(name="pgate", bufs=2, space="PSUM"))

            # ---- constants ----
            lw = {}
            bias = {}
            for t in range(NSLOT):
                lw[t] = consts.tile([P[t]["K"], P[t]["M"]], dt, tag=f"lw{t}", name=f"lw{t}")
                nc.sync.dma_start(out=lw[t], in_=dr[f"lw{t}"][:])
                bias[t] = consts.tile([128, 1], f32, tag=f"bias{t}", name=f"biast{t}")
                nc.sync.dma_start(out=bias[t], in_=dr[f"bias{t}"][:])
            hw = {}
            for name in ["whg", "who01", "who23", "who4", "w2a", "w2b", "w2c", "w3",
                         "iadd10", "iadd20", "iadd30"]:
                hw[name] = consts.tile(list(dr[name].shape), dt, tag=name, name=f"hw_{name}")
                nc.sync.dma_start(out=hw[name], in_=dr[name][:])
            hb = consts.tile([128, 8], f32, tag="hbias")
            nc.sync.dma_start(out=hb, in_=dr["hbias"][:])


            for it in range(n_tiles):
                col = slice(it * FD, (it + 1) * FD)
                # head concat tiles
                HG = hp_.tile([100, FD], dt, tag="HG")
                HO = {0: hp_.tile([80, FD], dt, tag="HO01", name=f"HO01_{it}"),
                      1: hp_.tile([80, FD], dt, tag="HO23", name=f"HO23_{it}"),
                      2: hp_.tile([40, FD], dt, tag="HO4", name=f"HO4_{it}")}
                rhs = {}
                for t in range(NSLOT + 1):
                    kt = P[t]["K"] if t < NSLOT else 10
                    rhs[t] = rhsp.tile([kt, FD], dt, tag="rhs", name=f"rhs_{it}_{t}")
                # stage first two state DMAs; the rest issue inside the loop
                for t in (0, 1):
                    nc.sync.dma_start(out=rhs[t][P[t]["hp"]:P[t]["K"], :],
                                      in_=dr[f"st{t}"][:, col])

                for t in range(NSLOT):
                    info = P[t]
                    ncell = info["nc"]
                    w = 10 * ncell
                    if t + 2 < NSLOT:
                        t2 = t + 2
                        nc.sync.dma_start(out=rhs[t2][P[t2]["hp"]:P[t2]["K"], :],
                                          in_=dr[f"st{t2}"][:, col])
                    S = sp.tile([128, FD], dt, tag="S", name=f"S_{it}_{t}")
                    Z = zp.tile([GF + 32, FD], dt, tag="Z", name=f"Z_{it}_{t}")
                    U = up.tile([GF + 32, FD], dt, tag="U", name=f"U_{it}_{t}")
                    T2 = cp.tile([GO + 32, FD], dt, tag="T2", name=f"T2_{it}_{t}")
                    # c states -> Z[32:32+w]
                    nc.sync.dma_start(out=Z[GF:GF + w, :], in_=dr[f"ct{t}"][:, col])
                    pt = pg.tile([128, FD], f32, tag="pg", name=f"pg_{it}_{t}")
                    for m in range(FD // N_MM):
                        mcol = slice(m * N_MM, (m + 1) * N_MM)
                        nc.tensor.matmul(pt[:, mcol], lw[t][:],
                                         rhs[t][0:info["K"], mcol],
                                         start=True, stop=True)
                    # tanh over ALL gate groups (i,f,o pre-halved on host)
                    nc.scalar.activation(S[0:128, :], pt[0:128, :],
                                         AF.Tanh, bias=bias[t][0:128])
                    # move tanh(g) next to c for the fused product
                    nc.vector.tensor_copy(Z[0:32, :], S[GG:GG + 32, :])
                    # U = (T_if + 1) * [g | c]
                    nc.vector.scalar_tensor_tensor(
                        U[0:GF + w], S[0:GF + w], 1.0, Z[0:GF + w],
                        mybir.AluOpType.add, mybir.AluOpType.mult)
                    # c2 = 0.5*(row + row+32) back into pt[0:w] (psum reuse)
                    iw = hw[f"iadd{w}"]
                    for m in range(FD // N_MM):
                        mcol = slice(m * N_MM, (m + 1) * N_MM)
                        nc.tensor.matmul(pt[0:w, mcol], iw[:],
                                         U[0:GF + w, mcol],
                                         start=True, stop=True)
                    # T2 = tanh(c2) at base GO (pairs with T_o)
                    nc.scalar.activation(T2[GO:GO + w, :], pt[0:w, :], AF.Tanh)
                    # h' = 2h = (T_o + 1) * tanh(c2) -> next slot rhs rows 0:w
                    nc.vector.scalar_tensor_tensor(
                        rhs[t + 1][0:w, :], S[GO:GO + w, :], 1.0, T2[GO:GO + w, :],
                        mybir.AluOpType.add, mybir.AluOpType.mult)
                    # copy h pieces into head concat tiles (SBUF->SBUF DMA)
                    hsrc = rhs[t + 1]
                    if SLOTS[t][0][0] == "g":
                        gi = SLOTS[t][0][1]
                        nc.sync.dma_start(out=HG[10 * gi:10 * gi + 10, :], in_=hsrc[0:10, :])
                    for k, c in enumerate(SLOTS[t]):
                        if c[0] == "o":
                            p, s = c[1], c[2]
                            pair = p // 2 if p < 4 else 2
                            drow = (20 * s + 10 * (p % 2)) if p < 4 else 10 * s
                            nc.sync.dma_start(out=HO[pair][drow:drow + 10, :],
                                              in_=hsrc[10 * k:10 * k + 10, :])

                # ---- heads ----
                F1 = fp.tile([50, FD], dt, tag="F1", name=f"F1_{it}")
                Fo = {0: fp.tile([40, FD], dt, tag="Fo01", name=f"Fo01_{it}"),
                      1: fp.tile([40, FD], dt, tag="Fo23", name=f"Fo23_{it}"),
                      2: fp.tile([20, FD], dt, tag="Fo4", name=f"Fo4_{it}")}
                F2 = fp.tile([10, FD], dt, tag="F2", name=f"F2_{it}")
                out_sb = outp.tile([1, FD], f32, tag="out", name=f"out_{it}")

                def head_mm(psname, pairs, nrow, bias_ap, Fdst):
                    p_ = pg.tile([128, FD], f32, tag="pg", name=psname)
                    for m in range(FD // N_MM):
                        mc = slice(m * N_MM, (m + 1) * N_MM)
                        for j, (lh, rh) in enumerate(pairs):
                            nc.tensor.matmul(p_[0:nrow, mc], lh[:], rh[:, mc],
                                             start=(j == 0), stop=(j == len(pairs) - 1))
                    nc.scalar.activation(Fdst[0:nrow, :], p_[0:nrow, :],
                                         AF.Tanh, bias=bias_ap)

                head_mm(f"p1_{it}", [(hw["whg"], HG)], 50, hb[0:50, 0:1], F1)
                head_mm(f"po1_{it}", [(hw["who01"], HO[0])], 40, hb[0:40, 1:2], Fo[0])
                head_mm(f"po2_{it}", [(hw["who23"], HO[1])], 40, hb[0:40, 1:2], Fo[1])
                head_mm(f"po3_{it}", [(hw["who4"], HO[2])], 20, hb[0:20, 2:3], Fo[2])
                head_mm(f"p2_{it}",
                        [(hw["w2a"], F1), (hw["w2b"], Fo[0]),
                         (hw["w2b"], Fo[1]), (hw["w2c"], Fo[2])],
                        10, hb[0:10, 3:4], F2)
                p3 = pg.tile([128, FD], f32, tag="pg", name=f"p3_{it}")
                for m in range(FD // N_MM):
                    mc = slice(m * N_MM, (m + 1) * N_MM)
                    nc.tensor.matmul(p3[0:1, mc], hw["w3"][:], F2[:, mc],
                                     start=True, stop=True)
                nc.scalar.activation(out_sb[0:1, :], p3[0:1, :],
                                     AF.Tanh, bias=hb[0:1, 4:5])
                nc.sync.dma_start(out=out_d[0:1, col], in_=out_sb)

    nc.finalize()
    return nc


def kernel(**inputs):
    import ml_dtypes
    np_dt = ml_dtypes.bfloat16
    FD = 2048
    inputs = {k: np.asarray(v) for k, v in inputs.items()}
    packed = pack_host(inputs, np_dt)
    nc = build_nc(BC, FD, np_dt)

    batch_keys = [k for k in packed if k.startswith(("st", "ct"))]
    in_maps = []
    for c in range(NCORE):
        m = {}
        for k, v in packed.items():
            if k in batch_keys:
                m[k] = np.ascontiguousarray(v[:, c * BC:(c + 1) * BC])
            else:
                m[k] = v
        in_maps.append(m)

    from concourse.bass_utils import run_bass_kernel_spmd
    res = run_bass_kernel_spmd(nc, in_maps, list(range(NCORE)))
    outs = [res.results[c]["out"].reshape(-1) for c in range(NCORE)]
    return np.concatenate(outs).reshape(B, 1).astype(np.float32)


if __name__ == "__main__":
    pass



# revision 46
# speedup vs baseline: 2.0222x; 2.0222x over previous
"""Trainium2 Bass kernel for nn_Net_6maxFull (batch of tiny LSTM chains).

V2.6 design (sigma formulation, 10-slot schedule, 8-way interleave, FD=1024):
  - 30 LSTM cells in 10 slots x 3 cells (gen chain + 2 opp lanes/slot;
    opp chains may skip slots -- h carried via rhs rows + late copies).
  - Gates matmul M-layout 32-aligned [ai|af|2ag|ao] (parity-swapped o/g
    for odd interleave lanes); ONE sigmoid act gives S = [i, f, g', o]
    with g' = sigma(2ag) = (tanh(ag)+1)/2.
  - Z = [g'(DVE copy) | c(DMA)]; U = S[0:64] * Z  (one 2x-mode TT) gives
    [P=i.g', FC=f.c]; 2c2 = 4P + 2FC - 2i via two accumulating matmuls
    (U with coeffs 4,2; S[0:32] with -2I).
  - act2 = Tanh(scale=0.5) over a PAIR-shared psum [2c2_odd | 2c2_even],
    output shifted +64 so each T2 lands at its tile's o base (64/96);
    h = T2 * o as a plain TT, alternating DVE / GpSimd(Pool).
  - Head concat: 1 DMA per slot into CC tiles [120 rows = 4 slots x 30];
    heads contract CC directly with rearranged weights; F2/out packed x3.
  - DMA routing: st/ct bulk loads on SP (HWDGE); concat/late copies on
    gpsimd (SWDGE cheap triggers).
  - DVE lane rules honored: elementwise operand pairs share a 32-aligned
    base partition and never straddle the 64-lane boundary.
"""
import sys
import numpy as np

sys.path.insert(0, "/opt/trn_rl_repo")

B = 131072
NCORE = 8
BC = B // NCORE
H = 10

# slot schedule: cell = ("g", layer) or ("o", branch, step)
SLOTS = [
    [("g", 0), ("o", 0, 0), ("o", 1, 0)],
    [("g", 1), ("o", 0, 1), ("o", 1, 1)],
    [("g", 2), ("o", 0, 2), ("o", 1, 2)],
    [("g", 3), ("o", 0, 3), ("o", 2, 0)],
    [("g", 4), ("o", 1, 3), ("o", 2, 1)],
    [("g", 5), ("o", 2, 2), ("o", 3, 0)],
    [("g", 6), ("o", 2, 3), ("o", 4, 0)],
    [("g", 7), ("o", 3, 1), ("o", 4, 1)],
    [("g", 8), ("o", 3, 2), ("o", 4, 2)],
    [("g", 9), ("o", 3, 3), ("o", 4, 3)],
]
NSLOT = len(SLOTS)
W3C = 30          # rows per slot of h output (3 cells x 10)


def _is_start(cell):
    return (cell[0] == "g" and cell[1] == 0) or (cell[0] == "o" and cell[2] == 0)


def _pred(cell):
    return ("g", cell[1] - 1) if cell[0] == "g" else ("o", cell[1], cell[2] - 1)


def _x_rows(cell):
    if cell[0] == "g":
        return (0, 12)
    p = cell[1]
    s = 12 + 5 * p + 1
    return (s, s + 4)


class Plan:
    """Host-side layout plan for rhs rows / lhsT maps / late copies."""

    def __init__(self):
        self.slot = []
        # for each slot, where did each cell's h go (slot_idx, row) --
        # h of slot t cell k lives at rhs[t+1] rows 10k:10k+10
        pos_in = {}      # cell -> (slot, pos)
        for t, cells in enumerate(SLOTS):
            for k, c in enumerate(cells):
                pos_in[c] = (t, k)
        self.pos_in = pos_in
        # late-copy edges: pred h not in immediately preceding slot
        # (src_rhs_tile = pred_slot+1, rows 10*pred_pos; dst rhs[t][30:40])
        self.late = {}   # t -> (src_slot+1, src_row)
        for t, cells in enumerate(SLOTS):
            for c in cells:
                if _is_start(c):
                    continue
                pt_, pk = pos_in[_pred(c)]
                if pt_ != t - 1:
                    assert t not in self.late, "only one late edge per slot"
                    self.late[t] = (pt_ + 1, 10 * pk)
        for t, cells in enumerate(SLOTS):
            info = {"cells": cells}
            info["hp"] = 0 if t == 0 else (40 if t in self.late else 30)
            rows = []        # (kind, cell): x rows for start cells
            for c in cells:
                if _is_start(c):
                    rows.append(("x", c))
            for c in cells:
                rows.append(("h", c))
            info["strows"] = rows
            nx = sum(12 if c[0] == "g" else 4 for k, c in rows if k == "x")
            info["R"] = nx + W3C              # x rows + h-state rows
            info["Kmm"] = info["hp"] + info["R"]
            assert info["Kmm"] <= 128
            self.slot.append(info)

    def chain_row(self, t, cell):
        """rhs[t] row where this chained cell's input h/2 lives."""
        pt_, pk = self.pos_in[_pred(cell)]
        if pt_ == t - 1:
            return 10 * pk
        return 30  # late-copy target rows


PLAN = Plan()


def _cell_w(inp, cell):
    if cell[0] == "g":
        i = cell[1]
        if i == 0:
            return (inp["W_g0_ih"], inp["W_g0_hh"], inp["b_g0_ih"] + inp["b_g0_hh"])
        return (inp["W_g_ih"][i - 1], inp["W_g_hh"][i - 1],
                inp["b_g_ih"][i - 1] + inp["b_g_hh"][i - 1])
    p, s = cell[1], cell[2]
    if s == 0:
        return (inp["W_o0_ih"][p], inp["W_o0_hh"][p],
                inp["b_o0_ih"][p] + inp["b_o0_hh"][p])
    return (inp["W_o_ih"][p][s - 1], inp["W_o_hh"][p][s - 1],
            inp["b_o_ih"][p][s - 1] + inp["b_o_hh"][p][s - 1])


# CC tile mapping: slot t -> (cc_idx, row_base)
def _cc_of(t):
    return t // 4, 30 * (t % 4)


def pack_host(inp, np_dt):
    """Build all DRAM-side arrays (full batch; shard columns later)."""
    f32 = np.float32
    out = {}
    Bt = inp["x"].shape[0]
    xT = np.ascontiguousarray(np.asarray(inp["x"], f32).T.astype(np_dt))   # [37,B]

    def state(cell):
        if cell[0] == "g":
            return (np.asarray(inp["gen_h"][cell[1]], f32).T,
                    np.asarray(inp["gen_c"][cell[1]], f32).T)
        return (np.asarray(inp["opp_h"][cell[1]][cell[2]], f32).T,
                np.asarray(inp["opp_c"][cell[1]][cell[2]], f32).T)

    # gate group slices in torch order i,f,g,o within the [40, din] weights
    GS = {"i": slice(0, 10), "f": slice(10, 20), "g": slice(20, 30), "o": slice(30, 40)}
    # two M-layout parities: even tiles o@96 (g'@64), odd tiles o@64 (g'@96)
    GCOLS = ({"i": 0, "f": 32, "g": 64, "o": 96},
             {"i": 0, "f": 32, "o": 64, "g": 96})
    GSC = {"i": 1.0, "f": 1.0, "g": 2.0, "o": 1.0}

    for t, info in enumerate(PLAN.slot):
        cells = info["cells"]
        # ---- st block [Kmm - hp, B]: x rows then h-state rows ----
        st = np.zeros((info["Kmm"] - info["hp"], Bt), np_dt)
        r = 0
        xrow_of = {}
        for kind, c in info["strows"]:
            if kind == "x":
                a, b = _x_rows(c)
                st[r:r + (b - a)] = xT[a:b]
                xrow_of[c] = r
                r += b - a
        hrow_of = {}
        for k, c in enumerate(cells):
            h0, c0 = state(c)
            st[r:r + 10] = h0.astype(np_dt)
            hrow_of[c] = r
            r += 10
        out[f"st{t}"] = st
        # ---- ct block [32, B] (2 zero pad rows) ----
        ct = np.zeros((32, Bt), np_dt)
        for k, c in enumerate(cells):
            h0, c0 = state(c)
            ct[10 * k:10 * k + 10] = c0.astype(np_dt)
        out[f"ct{t}"] = ct

        # ---- gates lhsT [Kmm, 128] + bias1 [128,1], per parity ----
        for par in (0, 1):
            GCOL = GCOLS[par]
            lw = np.zeros((info["Kmm"], 128), f32)
            b1 = np.zeros((128, 1), f32)
            for k, c in enumerate(cells):
                Wih, Whh, bvec = (np.asarray(a, f32) for a in _cell_w(inp, c))
                if _is_start(c):
                    r0 = info["hp"] + xrow_of[c]
                    din = Wih.shape[1]
                    for gn in "ifgo":
                        lw[r0:r0 + din, GCOL[gn] + 10 * k:GCOL[gn] + 10 * k + 10] = \
                            GSC[gn] * Wih[GS[gn]].T
                else:
                    r0 = PLAN.chain_row(t, c)
                    for gn in "ifgo":
                        lw[r0:r0 + 10, GCOL[gn] + 10 * k:GCOL[gn] + 10 * k + 10] = \
                            GSC[gn] * Wih[GS[gn]].T
                r0 = info["hp"] + hrow_of[c]
                for gn in "ifgo":
                    lw[r0:r0 + 10, GCOL[gn] + 10 * k:GCOL[gn] + 10 * k + 10] = \
                        GSC[gn] * Whh[GS[gn]].T
                    b1[GCOL[gn] + 10 * k:GCOL[gn] + 10 * k + 10, 0] = GSC[gn] * bvec[GS[gn]]
            out[f"lwg{t}_{par}"] = lw.astype(np_dt)
            out[f"bias{t}_{par}"] = b1

    # ---- iadd lhsTs: 2c2 = 4*P + 2*FC (from U) - 2*i (from S[0:32]) ----
    ia = np.zeros((64, 32), f32)
    for j in range(W3C):
        ia[j, j] = 4.0
        ia[32 + j, j] = 2.0
    out["iadda"] = ia.astype(np_dt)
    ib = np.zeros((32, 32), f32)
    for j in range(W3C):
        ib[j, j] = -2.0
    out["iaddb"] = ib.astype(np_dt)

    # ---- heads: contract CC tiles [120 rows = 4 slots x (g|opp|opp)] ----
    W1 = np.asarray(inp["W1"], f32)      # [50, 100]
    W1o = np.asarray(inp["W1o"], f32)    # [20, 40]
    W2 = np.asarray(inp["W2"], f32)      # [10, 70]
    W3 = np.asarray(inp["W3"], f32)      # [1, 10]
    # wh1: -> [F1(0:50) | zA(64:84) | zB(84:104)]; wh2: -> [zC|zD|zE] (60)
    for q in range(3):
        rows = 120 if q < 2 else 60
        w1q = np.zeros((rows, 104), f32)
        w2q = np.zeros((rows, 60), f32)
        for t in range(4 * q, min(4 * q + 4, NSLOT)):
            rb = 30 * (t % 4)
            for k, c in enumerate(SLOTS[t]):
                rr = rb + 10 * k
                if c[0] == "g":
                    w1q[rr:rr + 10, 0:50] = W1[:, 10 * c[1]:10 * c[1] + 10].T
                else:
                    p, s = c[1], c[2]
                    blk = W1o[:, 10 * s:10 * s + 10].T
                    if p < 2:
                        w1q[rr:rr + 10, 64 + 20 * p:64 + 20 * p + 20] = blk
                    else:
                        w2q[rr:rr + 10, 20 * (p - 2):20 * (p - 2) + 20] = blk
        out[f"wh1_{q}"] = w1q.astype(np_dt)
        out[f"wh2_{q}"] = w2q.astype(np_dt)
    w2o = (W2[:, 50:70] / 5.0).T                      # [20, 10]
    # 32 output cols (10 real + 22 zero) so the packed psF psum rows
    # 32j..32j+32 are all matmul-written (no stale-garbage rows feeding
    # the later block-diag p3 contraction).
    w2full = np.zeros((104, 32), f32)
    w2full[0:50, 0:10] = W2[:, 0:50].T
    w2full[64:104, 0:10] = np.vstack([w2o, w2o])
    out["w2full"] = w2full.astype(np_dt)
    w2cde = np.zeros((60, 32), f32)
    w2cde[:, 0:10] = np.vstack([w2o, w2o, w2o])
    out["w2cde"] = w2cde.astype(np_dt)
    w3blk = np.zeros((74, 3), f32)                    # block-diag W3 x3 tiles
    for j in range(3):
        w3blk[32 * j:32 * j + 10, j] = np.asarray(W3, f32)[0]
    out["w3blk"] = w3blk.astype(np_dt)
    hb = np.zeros((128, 4), f32)
    hb[0:50, 0] = np.asarray(inp["b1"], f32)
    hb[64:104, 0] = np.tile(np.asarray(inp["b1o"], f32), 2)
    hb[0:60, 1] = np.tile(np.asarray(inp["b1o"], f32), 3)
    for j in range(3):                                     # packed x3 at 32j
        hb[32 * j:32 * j + 10, 2] = np.asarray(inp["b2"], f32)
    hb[0:3, 3] = float(np.asarray(inp["b3"], f32)[0])
    out["hbias"] = hb
    return out


def build_nc(Bc, FD, np_dt, n_ilv=8):
    """SPMD Bass program for one core; n_ilv batch tiles interleaved."""
    import concourse.bass as bass
    import concourse.tile as tile
    from concourse import bacc, mybir

    dt = {np.dtype(np.float32): mybir.dt.float32}.get(np.dtype(np_dt))
    if dt is None:
        import ml_dtypes
        assert np.dtype(np_dt) == np.dtype(ml_dtypes.bfloat16)
        dt = mybir.dt.bfloat16
    f32 = mybir.dt.float32
    AF = mybir.ActivationFunctionType
    ALU = mybir.AluOpType

    NMM = 512                       # psum f32 bank cols
    n_tiles = Bc // FD
    assert Bc % FD == 0 and FD % NMM == 0
    nchunk = FD // NMM
    assert n_tiles % n_ilv == 0
    # act2/head pack groups (PE out base partition must be 0/32/64 -> max 3)
    GROUPS = [list(range(g, min(g + 3, n_ilv))) for g in range(0, n_ilv, 3)]

    nc = bacc.Bacc(None, target_bir_lowering=False, debug=False)
    P = PLAN.slot
    dr = {}
    for t in range(NSLOT):
        dr[f"st{t}"] = nc.declare_dram_parameter(f"st{t}", [P[t]["Kmm"] - P[t]["hp"], Bc], dt, isOutput=False)
        dr[f"ct{t}"] = nc.declare_dram_parameter(f"ct{t}", [32, Bc], dt, isOutput=False)
        for par in (0, 1):
            dr[f"lwg{t}_{par}"] = nc.declare_dram_parameter(f"lwg{t}_{par}", [P[t]["Kmm"], 128], dt, isOutput=False)
            dr[f"bias{t}_{par}"] = nc.declare_dram_parameter(f"bias{t}_{par}", [128, 1], f32, isOutput=False)
    dr["iadda"] = nc.declare_dram_parameter("iadda", [64, 32], dt, isOutput=False)
    dr["iaddb"] = nc.declare_dram_parameter("iaddb", [32, 32], dt, isOutput=False)
    for q in range(3):
        rows = 120 if q < 2 else 60
        dr[f"wh1_{q}"] = nc.declare_dram_parameter(f"wh1_{q}", [rows, 104], dt, isOutput=False)
        dr[f"wh2_{q}"] = nc.declare_dram_parameter(f"wh2_{q}", [rows, 60], dt, isOutput=False)
    for name, shp in [("w2full", [104, 32]), ("w2cde", [60, 32]), ("w3blk", [74, 3])]:
        dr[name] = nc.declare_dram_parameter(name, shp, dt, isOutput=False)
    dr["hbias"] = nc.declare_dram_parameter("hbias", [128, 4], f32, isOutput=False)
    out_d = nc.declare_dram_parameter("out", [1, Bc], f32, isOutput=True)

    from contextlib import ExitStack
    with tile.TileContext(nc) as tc:
        with ExitStack() as ctx:
            consts = ctx.enter_context(tc.tile_pool(name="consts", bufs=1))
            rhsp = ctx.enter_context(tc.tile_pool(name="rhs", bufs=2))
            sp = ctx.enter_context(tc.tile_pool(name="sS", bufs=1))
            up = ctx.enter_context(tc.tile_pool(name="uU", bufs=1))
            s2p = ctx.enter_context(tc.tile_pool(name="s2", bufs=2))
            ccp = ctx.enter_context(tc.tile_pool(name="cc", bufs=1))
            fhp = ctx.enter_context(tc.tile_pool(name="fh", bufs=2))
            outp = ctx.enter_context(tc.tile_pool(name="osb", bufs=1))
            pg = ctx.enter_context(tc.tile_pool(name="pgate", bufs=2, space="PSUM"))
            pc = ctx.enter_context(tc.tile_pool(name="pctl", bufs=2, space="PSUM"))

            # ---- constants ----
            lwg, bias = {}, {}
            for t in range(NSLOT):
                for par in (0, 1):
                    key = (t, par)
                    lwg[key] = consts.tile([P[t]["Kmm"], 128], dt,
                                           tag=f"lwg{t}_{par}", name=f"lwg{t}_{par}")
                    nc.sync.dma_start(out=lwg[key], in_=dr[f"lwg{t}_{par}"][:])
                    bias[key] = consts.tile([128, 1], f32,
                                            tag=f"bias{t}_{par}", name=f"bias{t}_{par}")
                    nc.sync.dma_start(out=bias[key], in_=dr[f"bias{t}_{par}"][:])
            iadda = consts.tile([64, 32], dt, tag="iadda", name="iadda")
            nc.sync.dma_start(out=iadda, in_=dr["iadda"][:])
            iaddb = consts.tile([32, 32], dt, tag="iaddb", name="iaddb")
            nc.sync.dma_start(out=iaddb, in_=dr["iaddb"][:])
            hw = {}
            for q in range(3):
                rows = 120 if q < 2 else 60
                for nm, ncol in (("wh1", 104), ("wh2", 60)):
                    key = f"{nm}_{q}"
                    hw[key] = consts.tile([rows, ncol], dt, tag=key, name=key)
                    nc.sync.dma_start(out=hw[key], in_=dr[key][:])
            for nm in ("w2full", "w2cde", "w3blk"):
                hw[nm] = consts.tile(list(dr[nm].shape), dt, tag=nm, name=nm)
                nc.sync.dma_start(out=hw[nm], in_=dr[nm][:])
            hb = consts.tile([128, 4], f32, tag="hbias")
            nc.sync.dma_start(out=hb, in_=dr["hbias"][:])

            # process tiles in octets
            for base in range(0, n_tiles, n_ilv):
                xs = list(range(base, base + n_ilv))
                col_of = {x: x * FD for x in xs}
                rhs = {}      # (x, t) -> tile
                S = {}
                CC = {}
                for x in xs:
                    for q in range(3):
                        rows = 120 if q < 2 else 60
                        CC[(x, q)] = ccp.tile([rows, FD], dt, tag=f"CC{x - base}_{q}",
                                              name=f"CC_{x}_{q}")

                def alloc_rhs(x, t):
                    if t > NSLOT:
                        return
                    if t == NSLOT:
                        rhs[(x, t)] = rhsp.tile([W3C, FD], dt, tag=f"rhs{x - base}",
                                                name=f"rhs_{x}_{t}")
                        return
                    rhs[(x, t)] = rhsp.tile([P[t]["Kmm"], FD], dt, tag=f"rhs{x - base}",
                                            name=f"rhs_{x}_{t}")

                def st_dma(x, t):
                    if t >= NSLOT:
                        return
                    info = P[t]
                    col = slice(col_of[x], col_of[x] + FD)
                    nc.sync.dma_start(out=rhs[(x, t)][info["hp"]:info["Kmm"], :],
                                      in_=dr[f"st{t}"][:, col])

                Zs = {}

                def alloc_z(x, t):
                    if t >= NSLOT:
                        return
                    Zs[(x, t)] = up.tile([64, FD], dt, tag=f"Z{x - base}", bufs=2,
                                         name=f"Z_{x}_{t}")
                    nc.sync.dma_start(out=Zs[(x, t)][32:64, :],
                                      in_=dr[f"ct{t}"][:, col_of[x]:col_of[x] + FD])

                for x in xs:
                    alloc_rhs(x, 0)
                    alloc_rhs(x, 1)
                    st_dma(x, 0)
                    alloc_z(x, 0)

                for t in range(NSLOT):
                    info = P[t]
                    # prefetch next slot's state rows
                    for x in xs:
                        alloc_rhs(x, t + 1)
                    if t + 1 < NSLOT:
                        for x in xs:
                            st_dma(x, t + 1)
                    for x in xs:
                        alloc_z(x, t + 1)
                    # gates matmul -> pt[0:128]; parity of x sets the M layout:
                    # even j: [ai|af|2ag|ao], odd j: [ai|af|ao|2ag]
                    pt = {}
                    for j, x in enumerate(xs):
                        pt[x] = pg.tile([128, FD], f32, tag="pt", name=f"pt_{x}_{t}")
                        for m in range(nchunk):
                            mc = slice(m * NMM, (m + 1) * NMM)
                            nc.tensor.matmul(pt[x][0:128, mc], lwg[(t, j % 2)][:],
                                             rhs[(x, t)][0:info["Kmm"], mc],
                                             start=True, stop=True)
                    # act1: sigma -> S
                    for j, x in enumerate(xs):
                        S[x] = sp.tile([128, FD], dt, tag=f"S{x - base}",
                                       name=f"S_{x}_{t}")
                        nc.scalar.activation(S[x][0:128, :], pt[x][0:128, :],
                                             AF.Sigmoid, bias=bias[(t, j % 2)][0:128])
                    # Z = [g'(copy) | c(dma'd earlier)];  U = [i*g' | f*c]
                    U = {}
                    for j, x in enumerate(xs):
                        Z = Zs[(x, t)]
                        gp_base = 64 if j % 2 == 0 else 96
                        nc.vector.tensor_copy(Z[0:32, :], S[x][gp_base:gp_base + 32, :])
                        U[x] = up.tile([64, FD], dt, tag=f"U{x - base}",
                                       name=f"U_{x}_{t}")
                        nc.vector.tensor_mul(U[x][0:64, :], S[x][0:64, :], Z[0:64, :])
                    # iadd (2 accumulating mms) -> pair-shared ptc;
                    # act2 = sigma, shifted +64 so T' lands at partner o's base
                    TP = {}
                    for qi in range(n_ilv // 2):
                        xa, xb = xs[2 * qi], xs[2 * qi + 1]   # even j, odd j
                        ptc = pc.tile([128, FD], f32, tag="ptc", name=f"ptc_{base}_{t}_{qi}")
                        for j2, x in ((1, xb), (0, xa)):      # odd -> rows 0:32
                            rb = 0 if j2 == 1 else 32
                            for m in range(nchunk):
                                mc = slice(m * NMM, (m + 1) * NMM)
                                nc.tensor.matmul(ptc[rb:rb + 32, mc], iadda[:],
                                                 U[x][0:64, mc], start=True, stop=False)
                                nc.tensor.matmul(ptc[rb:rb + 32, mc], iaddb[:],
                                                 S[x][0:32, mc], start=False, stop=True)
                        tp = s2p.tile([128, FD], dt, tag=f"S2{qi % 2}",
                                      name=f"S2_{base}_{t}_{qi}")
                        nc.scalar.activation(tp[64:128, :], ptc[0:64, :],
                                             AF.Tanh, scale=0.5)
                        TP[xb] = (tp, 64)     # odd tile: T2 @64, o @64
                        TP[xa] = (tp, 96)     # even tile: T2 @96, o @96
                    # h = tanh(c2) * o -> rhs[t+1][0:30]
                    for j, x in enumerate(xs):
                        tp, r0 = TP[x]
                        eng = nc.vector if (j % 2 == 0) else nc.gpsimd
                        eng.tensor_mul(rhs[(x, t + 1)][0:W3C, :],
                                       tp[r0:r0 + 30, :], S[x][r0:r0 + 30, :])
                    # concat copy + late copy (SWDGE on gpsimd)
                    ccq, rb = _cc_of(t)
                    for x in xs:
                        nc.gpsimd.dma_start(out=CC[(x, ccq)][rb:rb + 30, :],
                                            in_=rhs[(x, t + 1)][0:W3C, :])
                    if t + 1 in PLAN.late:
                        src_slot, src_row = PLAN.late[t + 1]
                        assert src_slot == t
                        for x in xs:
                            nc.gpsimd.dma_start(
                                out=rhs[(x, t + 1)][30:40, :],
                                in_=rhs[(x, t)][src_row:src_row + 10, :])

                # ---- heads (per group of 3 to keep ring usage acyclic) ----
                FH, FH2 = {}, {}
                for qi, grp in enumerate(GROUPS):
                    quad = [xs[g] for g in grp]
                    for x in quad:
                        psA = pc.tile([128, FD], f32, tag="ptc", name=f"psA_{x}")
                        for m in range(nchunk):
                            mc = slice(m * NMM, (m + 1) * NMM)
                            for q in range(3):
                                nc.tensor.matmul(psA[0:104, mc], hw[f"wh1_{q}"][:],
                                                 CC[(x, q)][:, mc],
                                                 start=(q == 0), stop=(q == 2))
                        FH[x] = fhp.tile([104, FD], dt, tag=f"FH{(x - base) % 2}",
                                         name=f"FH_{x}")
                        nc.scalar.activation(FH[x][0:104, :], psA[0:104, :],
                                             AF.Tanh, bias=hb[0:104, 0:1])
                        psB = pc.tile([128, FD], f32, tag="ptc", name=f"psB_{x}")
                        for m in range(nchunk):
                            mc = slice(m * NMM, (m + 1) * NMM)
                            for q in range(3):
                                nc.tensor.matmul(psB[0:60, mc], hw[f"wh2_{q}"][:],
                                                 CC[(x, q)][:, mc],
                                                 start=(q == 0), stop=(q == 2))
                        FH2[x] = fhp.tile([60, FD], dt, tag=f"FH2{(x - base) % 2}",
                                          name=f"FH2_{x}")
                        nc.scalar.activation(FH2[x][0:60, :], psB[0:60, :],
                                             AF.Tanh, bias=hb[0:60, 1:2])
                    psF = pc.tile([128, FD], f32, tag="ptc", name=f"psF_{base}_{qi}")
                    for j, x in enumerate(quad):
                        for m in range(nchunk):
                            mc = slice(m * NMM, (m + 1) * NMM)
                            nc.tensor.matmul(psF[32 * j:32 * j + 32, mc], hw["w2full"][:],
                                             FH[x][0:104, mc], start=True, stop=False)
                            nc.tensor.matmul(psF[32 * j:32 * j + 32, mc], hw["w2cde"][:],
                                             FH2[x][0:60, mc], start=False, stop=True)
                    nrF = 32 * (len(quad) - 1) + 10
                    F2q = fhp.tile([74, FD], dt, tag=f"F2{qi}", name=f"F2_{base}_{qi}")
                    nc.scalar.activation(F2q[0:nrF, :], psF[0:nrF, :],
                                         AF.Tanh, bias=hb[0:nrF, 2:3])
                    psO = pc.tile([128, FD], f32, tag="ptc", name=f"psO_{base}_{qi}")
                    for m in range(nchunk):
                        mc = slice(m * NMM, (m + 1) * NMM)
                        nc.tensor.matmul(psO[0:len(quad), mc], hw["w3blk"][0:nrF, 0:len(quad)],
                                         F2q[0:nrF, mc], start=True, stop=True)
                    outq = outp.tile([3, FD], f32, tag=f"out{qi}", name=f"outq_{base}_{qi}")
                    nc.scalar.activation(outq[0:len(quad), :], psO[0:len(quad), :],
                                         AF.Tanh, bias=hb[0:len(quad), 3:4])
                    for j, x in enumerate(quad):
                        nc.gpsimd.dma_start(
                            out=out_d[0:1, col_of[x]:col_of[x] + FD],
                            in_=outq[j:j + 1, :])

    nc.finalize()
    return nc


def kernel(**inputs):
    import ml_dtypes
    np_dt = ml_dtypes.bfloat16
    FD = 1024
    inputs = {k: np.asarray(v) for k, v in inputs.items()}
    packed = pack_host(inputs, np_dt)
    nc = build_nc(BC, FD, np_dt)

    batch_keys = [k for k in packed if k.startswith(("st", "ct"))]
    in_maps = []
    for c in range(NCORE):
        m = {}
        for k, v in packed.items():
            if k in batch_keys:
                m[k] = np.ascontiguousarray(v[:, c * BC:(c + 1) * BC])
            else:
                m[k] = v
        in_maps.append(m)

    from concourse.bass_utils import run_bass_kernel_spmd
    res = run_bass_kernel_spmd(nc, in_maps, list(range(NCORE)))
    outs = [res.results[c]["out"].reshape(-1) for c in range(NCORE)]
    return np.concatenate(outs).reshape(B, 1).astype(np.float32)


if __name__ == "__main__":
    pass


# revision 49
# speedup vs baseline: 2.0581x; 1.0178x over previous
"""Trainium2 Bass kernel for nn_Net_6maxFull (batch of tiny LSTM chains).

V2.6 design (sigma formulation, 10-slot schedule, 8-way interleave, FD=1024):
  - 30 LSTM cells in 10 slots x 3 cells (gen chain + 2 opp lanes/slot;
    opp chains may skip slots -- h carried via rhs rows + late copies).
  - Gates matmul M-layout 32-aligned [ai|af|2ag|ao] (parity-swapped o/g
    for odd interleave lanes); ONE sigmoid act gives S = [i, f, g', o]
    with g' = sigma(2ag) = (tanh(ag)+1)/2.
  - Z = [g'(DVE copy) | c(DMA)]; U = S[0:64] * Z  (one 2x-mode TT) gives
    [P=i.g', FC=f.c]; 2c2 = 4P + 2FC - 2i via two accumulating matmuls
    (U with coeffs 4,2; S[0:32] with -2I).
  - act2 = Tanh(scale=0.5) over a PAIR-shared psum [2c2_odd | 2c2_even],
    output shifted +64 so each T2 lands at its tile's o base (64/96);
    h = T2 * o as a plain TT, alternating DVE / GpSimd(Pool).
  - Head concat: 1 DMA per slot into CC tiles [120 rows = 4 slots x 30];
    heads contract CC directly with rearranged weights; F2/out packed x3.
  - DMA routing: st/ct bulk loads on SP (HWDGE); concat/late copies on
    gpsimd (SWDGE cheap triggers).
  - DVE lane rules honored: elementwise operand pairs share a 32-aligned
    base partition and never straddle the 64-lane boundary.
"""
import sys
import numpy as np

sys.path.insert(0, "/opt/trn_rl_repo")

B = 131072
NCORE = 8
BC = B // NCORE
H = 10

# slot schedule: cell = ("g", layer) or ("o", branch, step)
SLOTS = [
    [("g", 0), ("o", 0, 0), ("o", 1, 0)],
    [("g", 1), ("o", 0, 1), ("o", 1, 1)],
    [("g", 2), ("o", 0, 2), ("o", 1, 2)],
    [("g", 3), ("o", 0, 3), ("o", 2, 0)],
    [("g", 4), ("o", 1, 3), ("o", 2, 1)],
    [("g", 5), ("o", 2, 2), ("o", 3, 0)],
    [("g", 6), ("o", 2, 3), ("o", 4, 0)],
    [("g", 7), ("o", 3, 1), ("o", 4, 1)],
    [("g", 8), ("o", 3, 2), ("o", 4, 2)],
    [("g", 9), ("o", 3, 3), ("o", 4, 3)],
]
NSLOT = len(SLOTS)
W3C = 30          # rows per slot of h output (3 cells x 10)


def _is_start(cell):
    return (cell[0] == "g" and cell[1] == 0) or (cell[0] == "o" and cell[2] == 0)


def _pred(cell):
    return ("g", cell[1] - 1) if cell[0] == "g" else ("o", cell[1], cell[2] - 1)


def _x_rows(cell):
    if cell[0] == "g":
        return (0, 12)
    p = cell[1]
    s = 12 + 5 * p + 1
    return (s, s + 4)


class Plan:
    """Host-side layout plan for rhs rows / lhsT maps / late copies."""

    def __init__(self):
        self.slot = []
        # for each slot, where did each cell's h go (slot_idx, row) --
        # h of slot t cell k lives at rhs[t+1] rows 10k:10k+10
        pos_in = {}      # cell -> (slot, pos)
        for t, cells in enumerate(SLOTS):
            for k, c in enumerate(cells):
                pos_in[c] = (t, k)
        self.pos_in = pos_in
        # late-copy edges: pred h not in immediately preceding slot
        # (src_rhs_tile = pred_slot+1, rows 10*pred_pos; dst rhs[t][30:40])
        self.late = {}   # t -> (src_slot+1, src_row)
        for t, cells in enumerate(SLOTS):
            for c in cells:
                if _is_start(c):
                    continue
                pt_, pk = pos_in[_pred(c)]
                if pt_ != t - 1:
                    assert t not in self.late, "only one late edge per slot"
                    self.late[t] = (pt_ + 1, 10 * pk)
        for t, cells in enumerate(SLOTS):
            info = {"cells": cells}
            info["hp"] = 0 if t == 0 else (40 if t in self.late else 30)
            rows = []        # (kind, cell): x rows for start cells
            for c in cells:
                if _is_start(c):
                    rows.append(("x", c))
            for c in cells:
                rows.append(("h", c))
            info["strows"] = rows
            nx = sum(12 if c[0] == "g" else 4 for k, c in rows if k == "x")
            info["R"] = nx + W3C              # x rows + h-state rows
            info["Kmm"] = info["hp"] + info["R"]
            assert info["Kmm"] <= 128
            self.slot.append(info)

    def chain_row(self, t, cell):
        """rhs[t] row where this chained cell's input h/2 lives."""
        pt_, pk = self.pos_in[_pred(cell)]
        if pt_ == t - 1:
            return 10 * pk
        return 30  # late-copy target rows


PLAN = Plan()


def _cell_w(inp, cell):
    if cell[0] == "g":
        i = cell[1]
        if i == 0:
            return (inp["W_g0_ih"], inp["W_g0_hh"], inp["b_g0_ih"] + inp["b_g0_hh"])
        return (inp["W_g_ih"][i - 1], inp["W_g_hh"][i - 1],
                inp["b_g_ih"][i - 1] + inp["b_g_hh"][i - 1])
    p, s = cell[1], cell[2]
    if s == 0:
        return (inp["W_o0_ih"][p], inp["W_o0_hh"][p],
                inp["b_o0_ih"][p] + inp["b_o0_hh"][p])
    return (inp["W_o_ih"][p][s - 1], inp["W_o_hh"][p][s - 1],
            inp["b_o_ih"][p][s - 1] + inp["b_o_hh"][p][s - 1])


# CC tile mapping: slot t -> (cc_idx, row_base)
def _cc_of(t):
    return t // 4, 30 * (t % 4)


def pack_host(inp, np_dt):
    """Build all DRAM-side arrays (full batch; shard columns later)."""
    f32 = np.float32
    out = {}
    Bt = inp["x"].shape[0]
    xT = np.ascontiguousarray(np.asarray(inp["x"], f32).T.astype(np_dt))   # [37,B]

    def state(cell):
        if cell[0] == "g":
            return (np.asarray(inp["gen_h"][cell[1]], f32).T,
                    np.asarray(inp["gen_c"][cell[1]], f32).T)
        return (np.asarray(inp["opp_h"][cell[1]][cell[2]], f32).T,
                np.asarray(inp["opp_c"][cell[1]][cell[2]], f32).T)

    # gate group slices in torch order i,f,g,o within the [40, din] weights
    GS = {"i": slice(0, 10), "f": slice(10, 20), "g": slice(20, 30), "o": slice(30, 40)}
    # two M-layout parities: even tiles o@96 (g'@64), odd tiles o@64 (g'@96)
    GCOLS = ({"i": 0, "f": 32, "g": 64, "o": 96},
             {"i": 0, "f": 32, "o": 64, "g": 96})
    GSC = {"i": 1.0, "f": 1.0, "g": 2.0, "o": 1.0}

    for t, info in enumerate(PLAN.slot):
        cells = info["cells"]
        # ---- st block [Kmm - hp, B]: x rows then h-state rows ----
        st = np.zeros((info["Kmm"] - info["hp"], Bt), np_dt)
        r = 0
        xrow_of = {}
        for kind, c in info["strows"]:
            if kind == "x":
                a, b = _x_rows(c)
                st[r:r + (b - a)] = xT[a:b]
                xrow_of[c] = r
                r += b - a
        hrow_of = {}
        for k, c in enumerate(cells):
            h0, c0 = state(c)
            st[r:r + 10] = h0.astype(np_dt)
            hrow_of[c] = r
            r += 10
        out[f"st{t}"] = st
        # ---- ct block [32, B] (2 zero pad rows) ----
        ct = np.zeros((32, Bt), np_dt)
        for k, c in enumerate(cells):
            h0, c0 = state(c)
            ct[10 * k:10 * k + 10] = c0.astype(np_dt)
        out[f"ct{t}"] = ct

        # ---- gates lhsT [Kmm, 128] + bias1 [128,1], per parity ----
        for par in (0, 1):
            GCOL = GCOLS[par]
            lw = np.zeros((info["Kmm"], 128), f32)
            b1 = np.zeros((128, 1), f32)
            for k, c in enumerate(cells):
                Wih, Whh, bvec = (np.asarray(a, f32) for a in _cell_w(inp, c))
                if _is_start(c):
                    r0 = info["hp"] + xrow_of[c]
                    din = Wih.shape[1]
                    for gn in "ifgo":
                        lw[r0:r0 + din, GCOL[gn] + 10 * k:GCOL[gn] + 10 * k + 10] = \
                            GSC[gn] * Wih[GS[gn]].T
                else:
                    r0 = PLAN.chain_row(t, c)
                    for gn in "ifgo":
                        lw[r0:r0 + 10, GCOL[gn] + 10 * k:GCOL[gn] + 10 * k + 10] = \
                            GSC[gn] * Wih[GS[gn]].T
                r0 = info["hp"] + hrow_of[c]
                for gn in "ifgo":
                    lw[r0:r0 + 10, GCOL[gn] + 10 * k:GCOL[gn] + 10 * k + 10] = \
                        GSC[gn] * Whh[GS[gn]].T
                    b1[GCOL[gn] + 10 * k:GCOL[gn] + 10 * k + 10, 0] = GSC[gn] * bvec[GS[gn]]
            out[f"lwg{t}_{par}"] = lw.astype(np_dt)
            out[f"bias{t}_{par}"] = b1

    # ---- iadd lhsTs: 2c2 = 4*P + 2*FC (from U) - 2*i (from S[0:32]) ----
    ia = np.zeros((64, 32), f32)
    for j in range(W3C):
        ia[j, j] = 4.0
        ia[32 + j, j] = 2.0
    out["iadda"] = ia.astype(np_dt)
    ib = np.zeros((32, 32), f32)
    for j in range(W3C):
        ib[j, j] = -2.0
    out["iaddb"] = ib.astype(np_dt)

    # ---- heads: contract CC tiles [120 rows = 4 slots x (g|opp|opp)] ----
    W1 = np.asarray(inp["W1"], f32)      # [50, 100]
    W1o = np.asarray(inp["W1o"], f32)    # [20, 40]
    W2 = np.asarray(inp["W2"], f32)      # [10, 70]
    W3 = np.asarray(inp["W3"], f32)      # [1, 10]
    # wh1: -> [F1(0:50) | zA(64:84) | zB(84:104)]; wh2: -> [zC|zD|zE] (60)
    for q in range(3):
        rows = 120 if q < 2 else 60
        w1q = np.zeros((rows, 104), f32)
        w2q = np.zeros((rows, 60), f32)
        for t in range(4 * q, min(4 * q + 4, NSLOT)):
            rb = 30 * (t % 4)
            for k, c in enumerate(SLOTS[t]):
                rr = rb + 10 * k
                if c[0] == "g":
                    w1q[rr:rr + 10, 0:50] = W1[:, 10 * c[1]:10 * c[1] + 10].T
                else:
                    p, s = c[1], c[2]
                    blk = W1o[:, 10 * s:10 * s + 10].T
                    if p < 2:
                        w1q[rr:rr + 10, 64 + 20 * p:64 + 20 * p + 20] = blk
                    else:
                        w2q[rr:rr + 10, 20 * (p - 2):20 * (p - 2) + 20] = blk
        out[f"wh1_{q}"] = w1q.astype(np_dt)
        out[f"wh2_{q}"] = w2q.astype(np_dt)
    w2o = (W2[:, 50:70] / 5.0).T                      # [20, 10]
    # 32 output cols (10 real + 22 zero) so the packed psF psum rows
    # 32j..32j+32 are all matmul-written (no stale-garbage rows feeding
    # the later block-diag p3 contraction).
    w2full = np.zeros((104, 32), f32)
    w2full[0:50, 0:10] = W2[:, 0:50].T
    w2full[64:104, 0:10] = np.vstack([w2o, w2o])
    out["w2full"] = w2full.astype(np_dt)
    w2cde = np.zeros((60, 32), f32)
    w2cde[:, 0:10] = np.vstack([w2o, w2o, w2o])
    out["w2cde"] = w2cde.astype(np_dt)
    w3blk = np.zeros((74, 3), f32)                    # block-diag W3 x3 tiles
    for j in range(3):
        w3blk[32 * j:32 * j + 10, j] = np.asarray(W3, f32)[0]
    out["w3blk"] = w3blk.astype(np_dt)
    hb = np.zeros((128, 4), f32)
    hb[0:50, 0] = np.asarray(inp["b1"], f32)
    hb[64:104, 0] = np.tile(np.asarray(inp["b1o"], f32), 2)
    hb[0:60, 1] = np.tile(np.asarray(inp["b1o"], f32), 3)
    for j in range(3):                                     # packed x3 at 32j
        hb[32 * j:32 * j + 10, 2] = np.asarray(inp["b2"], f32)
    hb[0:3, 3] = float(np.asarray(inp["b3"], f32)[0])
    out["hbias"] = hb
    return out


def build_nc(Bc, FD, np_dt, n_ilv=8):
    """SPMD Bass program for one core; n_ilv batch tiles interleaved."""
    import concourse.bass as bass
    import concourse.tile as tile
    from concourse import bacc, mybir

    dt = {np.dtype(np.float32): mybir.dt.float32}.get(np.dtype(np_dt))
    if dt is None:
        import ml_dtypes
        assert np.dtype(np_dt) == np.dtype(ml_dtypes.bfloat16)
        dt = mybir.dt.bfloat16
    f32 = mybir.dt.float32
    AF = mybir.ActivationFunctionType
    ALU = mybir.AluOpType

    NMM = 512                       # psum f32 bank cols
    n_tiles = Bc // FD
    assert Bc % FD == 0 and FD % NMM == 0
    nchunk = FD // NMM
    assert n_tiles % n_ilv == 0
    # act2/head pack groups (PE out base partition must be 0/32/64 -> max 3)
    GROUPS = [list(range(g, min(g + 3, n_ilv))) for g in range(0, n_ilv, 3)]

    nc = bacc.Bacc(None, target_bir_lowering=False, debug=False)
    P = PLAN.slot
    dr = {}
    for t in range(NSLOT):
        dr[f"st{t}"] = nc.declare_dram_parameter(f"st{t}", [P[t]["Kmm"] - P[t]["hp"], Bc], dt, isOutput=False)
        dr[f"ct{t}"] = nc.declare_dram_parameter(f"ct{t}", [32, Bc], dt, isOutput=False)
        for par in (0, 1):
            dr[f"lwg{t}_{par}"] = nc.declare_dram_parameter(f"lwg{t}_{par}", [P[t]["Kmm"], 128], dt, isOutput=False)
            dr[f"bias{t}_{par}"] = nc.declare_dram_parameter(f"bias{t}_{par}", [128, 1], f32, isOutput=False)
    dr["iadda"] = nc.declare_dram_parameter("iadda", [64, 32], dt, isOutput=False)
    dr["iaddb"] = nc.declare_dram_parameter("iaddb", [32, 32], dt, isOutput=False)
    for q in range(3):
        rows = 120 if q < 2 else 60
        dr[f"wh1_{q}"] = nc.declare_dram_parameter(f"wh1_{q}", [rows, 104], dt, isOutput=False)
        dr[f"wh2_{q}"] = nc.declare_dram_parameter(f"wh2_{q}", [rows, 60], dt, isOutput=False)
    for name, shp in [("w2full", [104, 32]), ("w2cde", [60, 32]), ("w3blk", [74, 3])]:
        dr[name] = nc.declare_dram_parameter(name, shp, dt, isOutput=False)
    dr["hbias"] = nc.declare_dram_parameter("hbias", [128, 4], f32, isOutput=False)
    out_d = nc.declare_dram_parameter("out", [1, Bc], f32, isOutput=True)

    from contextlib import ExitStack
    with tile.TileContext(nc) as tc:
        with ExitStack() as ctx:
            consts = ctx.enter_context(tc.tile_pool(name="consts", bufs=1))
            rhsp = ctx.enter_context(tc.tile_pool(name="rhs", bufs=2))
            sp = ctx.enter_context(tc.tile_pool(name="sS", bufs=1))
            up = ctx.enter_context(tc.tile_pool(name="uU", bufs=1))
            s2p = ctx.enter_context(tc.tile_pool(name="s2", bufs=2))
            ccp = ctx.enter_context(tc.tile_pool(name="cc", bufs=1))
            fhp = ctx.enter_context(tc.tile_pool(name="fh", bufs=2))
            outp = ctx.enter_context(tc.tile_pool(name="osb", bufs=1))
            pg = ctx.enter_context(tc.tile_pool(name="pgate", bufs=4, space="PSUM"))

            # ---- constants ----
            lwg, bias = {}, {}
            for t in range(NSLOT):
                for par in (0, 1):
                    key = (t, par)
                    lwg[key] = consts.tile([P[t]["Kmm"], 128], dt,
                                           tag=f"lwg{t}_{par}", name=f"lwg{t}_{par}")
                    nc.sync.dma_start(out=lwg[key], in_=dr[f"lwg{t}_{par}"][:])
                    bias[key] = consts.tile([128, 1], f32,
                                            tag=f"bias{t}_{par}", name=f"bias{t}_{par}")
                    nc.sync.dma_start(out=bias[key], in_=dr[f"bias{t}_{par}"][:])
            iadda = consts.tile([64, 32], dt, tag="iadda", name="iadda")
            nc.sync.dma_start(out=iadda, in_=dr["iadda"][:])
            iaddb = consts.tile([32, 32], dt, tag="iaddb", name="iaddb")
            nc.sync.dma_start(out=iaddb, in_=dr["iaddb"][:])
            hw = {}
            for q in range(3):
                rows = 120 if q < 2 else 60
                for nm, ncol in (("wh1", 104), ("wh2", 60)):
                    key = f"{nm}_{q}"
                    hw[key] = consts.tile([rows, ncol], dt, tag=key, name=key)
                    nc.sync.dma_start(out=hw[key], in_=dr[key][:])
            for nm in ("w2full", "w2cde", "w3blk"):
                hw[nm] = consts.tile(list(dr[nm].shape), dt, tag=nm, name=nm)
                nc.sync.dma_start(out=hw[nm], in_=dr[nm][:])
            hb = consts.tile([128, 4], f32, tag="hbias")
            nc.sync.dma_start(out=hb, in_=dr["hbias"][:])

            # process tiles in octets
            for base in range(0, n_tiles, n_ilv):
                xs = list(range(base, base + n_ilv))
                col_of = {x: x * FD for x in xs}
                rhs = {}      # (x, t) -> tile
                S = {}
                CC = {}
                for x in xs:
                    for q in range(3):
                        rows = 120 if q < 2 else 60
                        CC[(x, q)] = ccp.tile([rows, FD], dt, tag=f"CC{x - base}_{q}",
                                              name=f"CC_{x}_{q}")

                def alloc_rhs(x, t):
                    if t > NSLOT:
                        return
                    if t == NSLOT:
                        rhs[(x, t)] = rhsp.tile([W3C, FD], dt, tag=f"rhs{x - base}",
                                                name=f"rhs_{x}_{t}")
                        return
                    rhs[(x, t)] = rhsp.tile([P[t]["Kmm"], FD], dt, tag=f"rhs{x - base}",
                                            name=f"rhs_{x}_{t}")

                def st_dma(x, t):
                    if t >= NSLOT:
                        return
                    info = P[t]
                    col = slice(col_of[x], col_of[x] + FD)
                    nc.sync.dma_start(out=rhs[(x, t)][info["hp"]:info["Kmm"], :],
                                      in_=dr[f"st{t}"][:, col])

                Zs = {}

                def alloc_z(x, t):
                    if t >= NSLOT:
                        return
                    Zs[(x, t)] = up.tile([64, FD], dt, tag=f"Z{x - base}", bufs=2,
                                         name=f"Z_{x}_{t}")
                    nc.sync.dma_start(out=Zs[(x, t)][32:64, :],
                                      in_=dr[f"ct{t}"][:, col_of[x]:col_of[x] + FD])

                for x in xs:
                    alloc_rhs(x, 0)
                    alloc_rhs(x, 1)
                    st_dma(x, 0)
                    alloc_z(x, 0)

                for t in range(NSLOT):
                    info = P[t]
                    # prefetch next slot's state rows
                    for x in xs:
                        alloc_rhs(x, t + 1)
                    if t + 1 < NSLOT:
                        for x in xs:
                            st_dma(x, t + 1)
                    for x in xs:
                        alloc_z(x, t + 1)
                    # gates matmul -> pt[0:128]; parity of x sets the M layout:
                    # even j: [ai|af|2ag|ao], odd j: [ai|af|ao|2ag]
                    pt = {}
                    for j, x in enumerate(xs):
                        pt[x] = pg.tile([128, FD], f32, tag="pt", name=f"pt_{x}_{t}")
                        for m in range(nchunk):
                            mc = slice(m * NMM, (m + 1) * NMM)
                            nc.tensor.matmul(pt[x][0:128, mc], lwg[(t, j % 2)][:],
                                             rhs[(x, t)][0:info["Kmm"], mc],
                                             start=True, stop=True)
                    # act1: sigma -> S
                    for j, x in enumerate(xs):
                        S[x] = sp.tile([128, FD], dt, tag=f"S{x - base}",
                                       name=f"S_{x}_{t}")
                        nc.scalar.activation(S[x][0:128, :], pt[x][0:128, :],
                                             AF.Sigmoid, bias=bias[(t, j % 2)][0:128])
                    # Z = [g'(copy) | c(dma'd earlier)];  U = [i*g' | f*c]
                    U = {}
                    for j, x in enumerate(xs):
                        Z = Zs[(x, t)]
                        gp_base = 64 if j % 2 == 0 else 96
                        nc.vector.tensor_copy(Z[0:32, :], S[x][gp_base:gp_base + 32, :])
                        U[x] = up.tile([64, FD], dt, tag=f"U{x - base}",
                                       name=f"U_{x}_{t}")
                        nc.vector.tensor_mul(U[x][0:64, :], S[x][0:64, :], Z[0:64, :])
                    # iadd (2 accumulating mms) -> recycled pt[xb] rows 0:64;
                    # act2 = tanh(c2), shifted +64 so T2 lands at partner o's base
                    TP = {}
                    for qi in range(n_ilv // 2):
                        xa, xb = xs[2 * qi], xs[2 * qi + 1]   # even j, odd j
                        ptc = pt[xb]
                        for j2, x in ((1, xb), (0, xa)):      # odd -> rows 0:32
                            rb = 0 if j2 == 1 else 32
                            for m in range(nchunk):
                                mc = slice(m * NMM, (m + 1) * NMM)
                                nc.tensor.matmul(ptc[rb:rb + 32, mc], iadda[:],
                                                 U[x][0:64, mc], start=True, stop=False)
                                nc.tensor.matmul(ptc[rb:rb + 32, mc], iaddb[:],
                                                 S[x][0:32, mc], start=False, stop=True)
                        tp = s2p.tile([128, FD], dt, tag=f"S2{qi % 2}",
                                      name=f"S2_{base}_{t}_{qi}")
                        nc.scalar.activation(tp[64:128, :], ptc[0:64, :],
                                             AF.Tanh, scale=0.5)
                        TP[xb] = (tp, 64)     # odd tile: T2 @64, o @64
                        TP[xa] = (tp, 96)     # even tile: T2 @96, o @96
                    # h = tanh(c2) * o -> rhs[t+1][0:30]
                    for j, x in enumerate(xs):
                        tp, r0 = TP[x]
                        eng = nc.gpsimd if (j % 4 == 1) else nc.vector
                        eng.tensor_mul(rhs[(x, t + 1)][0:W3C, :],
                                       tp[r0:r0 + 30, :], S[x][r0:r0 + 30, :])
                    # concat copy + late copy (SWDGE on gpsimd)
                    ccq, rb = _cc_of(t)
                    for x in xs:
                        nc.gpsimd.dma_start(out=CC[(x, ccq)][rb:rb + 30, :],
                                            in_=rhs[(x, t + 1)][0:W3C, :])
                    if t + 1 in PLAN.late:
                        src_slot, src_row = PLAN.late[t + 1]
                        assert src_slot == t
                        for x in xs:
                            nc.gpsimd.dma_start(
                                out=rhs[(x, t + 1)][30:40, :],
                                in_=rhs[(x, t)][src_row:src_row + 10, :])

                # ---- heads (per group of 3 to keep ring usage acyclic) ----
                FH, FH2 = {}, {}
                for qi, grp in enumerate(GROUPS):
                    quad = [xs[g] for g in grp]
                    for x in quad:
                        psA = pg.tile([128, FD], f32, tag="pt", name=f"psA_{x}")
                        for m in range(nchunk):
                            mc = slice(m * NMM, (m + 1) * NMM)
                            for q in range(3):
                                nc.tensor.matmul(psA[0:104, mc], hw[f"wh1_{q}"][:],
                                                 CC[(x, q)][:, mc],
                                                 start=(q == 0), stop=(q == 2))
                        FH[x] = fhp.tile([104, FD], dt, tag=f"FH{(x - base) % 2}",
                                         name=f"FH_{x}")
                        nc.scalar.activation(FH[x][0:104, :], psA[0:104, :],
                                             AF.Tanh, bias=hb[0:104, 0:1])
                        psB = pg.tile([128, FD], f32, tag="pt", name=f"psB_{x}")
                        for m in range(nchunk):
                            mc = slice(m * NMM, (m + 1) * NMM)
                            for q in range(3):
                                nc.tensor.matmul(psB[0:60, mc], hw[f"wh2_{q}"][:],
                                                 CC[(x, q)][:, mc],
                                                 start=(q == 0), stop=(q == 2))
                        FH2[x] = fhp.tile([60, FD], dt, tag=f"FH2{(x - base) % 2}",
                                          name=f"FH2_{x}")
                        nc.scalar.activation(FH2[x][0:60, :], psB[0:60, :],
                                             AF.Tanh, bias=hb[0:60, 1:2])
                    psF = pg.tile([128, FD], f32, tag="pt", name=f"psF_{base}_{qi}")
                    for j, x in enumerate(quad):
                        for m in range(nchunk):
                            mc = slice(m * NMM, (m + 1) * NMM)
                            nc.tensor.matmul(psF[32 * j:32 * j + 32, mc], hw["w2full"][:],
                                             FH[x][0:104, mc], start=True, stop=False)
                            nc.tensor.matmul(psF[32 * j:32 * j + 32, mc], hw["w2cde"][:],
                                             FH2[x][0:60, mc], start=False, stop=True)
                    nrF = 32 * (len(quad) - 1) + 10
                    F2q = fhp.tile([74, FD], dt, tag=f"F2{qi}", name=f"F2_{base}_{qi}")
                    nc.scalar.activation(F2q[0:nrF, :], psF[0:nrF, :],
                                         AF.Tanh, bias=hb[0:nrF, 2:3])
                    psO = pg.tile([128, FD], f32, tag="pt", name=f"psO_{base}_{qi}")
                    for m in range(nchunk):
                        mc = slice(m * NMM, (m + 1) * NMM)
                        nc.tensor.matmul(psO[0:len(quad), mc], hw["w3blk"][0:nrF, 0:len(quad)],
                                         F2q[0:nrF, mc], start=True, stop=True)
                    outq = outp.tile([3, FD], f32, tag=f"out{qi}", name=f"outq_{base}_{qi}")
                    nc.scalar.activation(outq[0:len(quad), :], psO[0:len(quad), :],
                                         AF.Tanh, bias=hb[0:len(quad), 3:4])
                    for j, x in enumerate(quad):
                        nc.gpsimd.dma_start(
                            out=out_d[0:1, col_of[x]:col_of[x] + FD],
                            in_=outq[j:j + 1, :])

    nc.finalize()
    return nc


def kernel(**inputs):
    import ml_dtypes
    np_dt = ml_dtypes.bfloat16
    FD = 1024
    inputs = {k: np.asarray(v) for k, v in inputs.items()}
    packed = pack_host(inputs, np_dt)
    nc = build_nc(BC, FD, np_dt)

    batch_keys = [k for k in packed if k.startswith(("st", "ct"))]
    in_maps = []
    for c in range(NCORE):
        m = {}
        for k, v in packed.items():
            if k in batch_keys:
                m[k] = np.ascontiguousarray(v[:, c * BC:(c + 1) * BC])
            else:
                m[k] = v
        in_maps.append(m)

    from concourse.bass_utils import run_bass_kernel_spmd
    res = run_bass_kernel_spmd(nc, in_maps, list(range(NCORE)))
    outs = [res.results[c]["out"].reshape(-1) for c in range(NCORE)]
    return np.concatenate(outs).reshape(B, 1).astype(np.float32)


if __name__ == "__main__":
    pass
